# revision 4
# baseline (speedup 1.0000x reference)
"""ALiBi causal attention (B=2, S=2048, D=1024, H=16) on 8 TRN2 NeuronCores.

Sharding: core c owns batch c//4 and heads {j, 4+j, 8+j, 12+j} (j = c%4),
one head per "slot" 0..3. All cores run the same graph (SPMD); slot s uses
the ALiBi window of its shallowest head (h = 4s+3), so per-slot k-tile
lists are core-independent.

Per core:
  - x^T streamed in 4 sequence-quarters; Q^T/K^T projected with quarter-wide
    matmuls (4 psum sub-slices of one 4-bank tile), V projected directly in
    natural [s, d] layout (stationary = x^T slice) - no PE transposes.
  - windowed causal flash-attention per slot, two slot-streams round-robin
    per q-chunk; ALiBi bias folded into the scores matmul via 5 exact
    bf16 "augmented" rows; exp on ScalarE in quad-tile ACTIVATEs
    ([128,4,512], amortizes the ~352-cycle fixed cost); diagonal quad
    masked post-exp by a min with {1e9,0}; P^T@V accumulated with a
    ones-column in V producing the softmax denominator; 1/rowsum via DVE
    reciprocal on the replicated rows, DMA-shifted to base partitions.
  - W_O slice applied per 128-row tile, woven between attention groups of
    the next chunk to fill PE gaps while ScalarE runs exp.
Host sums the 4 partial outputs per batch (the W_O reduce) and reshapes.
"""

import math
import sys

sys.path.insert(0, "/opt/trn_rl_repo")

import ml_dtypes
import numpy as np

import concourse.bass as bass
import concourse.tile as tile
from concourse import bacc, mybir
from concourse.bass_utils import run_bass_kernel_spmd

B, S, D, H, HD = 2, 2048, 1024, 16, 64
P = 128          # k-tile / partition size
QC = 512         # q-chunk size
NKT = S // P     # 16 k-tiles
NQC = S // QC    # 4 q-chunks
NCORES = 8
T_WIN = 12.0     # ln cutoff: drop k-tiles with alibi penalty > T_WIN
VW = P           # V cols per (k-tile, slot): 64 data + 64 ones (the ones
                 # block makes P^T@V replicate the softmax denominator into
                 # psum rows 64:128, so no partition-broadcast is needed)

F32 = mybir.dt.float32
BF16 = mybir.dt.bfloat16

# slopes m_h = 2^-(h+1)/2, snapped to bf16 so every aug-row value is exactly
# representable regardless of PE input rounding.
SLOPES = np.float32(ml_dtypes.bfloat16(2.0 ** (-(np.arange(H, dtype=np.float64) + 1) / 2.0)))

# slot s covers heads 4s..4s+3 across the 4 cores of a batch; its k-tile
# window is set by the shallowest member (h = 4s+3). Slot 3 is full causal.
SLOT_L = [T_WIN / float(SLOPES[4 * s + 3]) for s in range(3)] + [None]


def _tiles_for_chunk(i: int, s: int):
    """k-tiles kept for q-chunk i of slot s (identical on all cores)."""
    if SLOT_L[s] is None:
        lo = 0
    else:
        lo = max(0, math.ceil((QC * i - SLOT_L[s] - (P - 1)) / P))
        if (4 * i + 4 - lo) % 2:  # keep counts even for pair/quad grouping
            lo = max(0, lo - 1)
    return list(range(lo, 4 * i + 4))


def _groups(lst):
    """Split an even-length ascending tile list into [pair?] + quads, so the
    final group is always the 4 diagonal tiles (or the whole list if len 4)."""
    out = []
    pos = 0
    if len(lst) % 4 == 2:
        out.append(lst[0:2])
        pos = 2
    while pos < len(lst):
        out.append(lst[pos:pos + 4])
        pos += 4
    return out


def _only_combined_act_table(arch):
    """Restrict the ACT LUT chooser so exp/copy resolve only to the one set
    holding them all - the Scalar engine never reloads tables mid-kernel."""
    from concourse import mybir as _mb
    from concourse.hw_specs import get_activation_tables as _orig
    tabs = _orig(arch)
    keep = "natural_log_exp_and_others"
    mine = {_mb.ActivationFunctionType.Exp, _mb.ActivationFunctionType.Ln,
            _mb.ActivationFunctionType.Copy, _mb.ActivationFunctionType.Identity}
    return {
        name: (fns if name == keep else (set(fns) - mine))
        for name, fns in tabs.items()
    }


def build_graph() -> bass.Bass:
    bacc.get_activation_tables = _only_combined_act_table
    nc = bacc.Bacc("TRN2", target_bir_lowering=False, debug=False)

    xTd = nc.dram_tensor("xT", (D, S), BF16, kind="ExternalInput").ap()
    wq = nc.dram_tensor("wq", (D, 256), BF16, kind="ExternalInput").ap()
    wk = nc.dram_tensor("wk", (D, 256), BF16, kind="ExternalInput").ap()
    wv = nc.dram_tensor("wv", (D, 256), BF16, kind="ExternalInput").ap()
    wo = nc.dram_tensor("wo", (256, D), BF16, kind="ExternalInput").ap()
    kaug = nc.dram_tensor("kaug", (4, 5, S), BF16, kind="ExternalInput").ap()
    qaug = nc.dram_tensor("qaug", (4, 5, S), BF16, kind="ExternalInput").ap()
    maskmin = nc.dram_tensor("maskmin", (4, P, QC), BF16, kind="ExternalInput").ap()
    out = nc.dram_tensor("out", (S, D), BF16, kind="ExternalOutput").ap()

    with tile.TileContext(nc) as tc:
        with (
            tc.tile_pool(name="sb", bufs=1) as sb,
            tc.tile_pool(name="ps", bufs=1, space="PSUM") as ps,
        ):
            # ---- persistent SBUF ----
            wq_sb = sb.tile([P, 8, 256], BF16, tag="wq")
            wk_sb = sb.tile([P, 8, 256], BF16, tag="wk")
            wv_sb = sb.tile([P, 8, 256], BF16, tag="wv")
            wo_sb = sb.tile([P, 2, D], BF16, tag="wo")
            mm_sb = sb.tile([P, 4, QC], BF16, tag="mask")
            QT = [sb.tile([69, S], BF16, tag=f"qt{s}", name=f"QT{s}") for s in range(4)]
            KT = [sb.tile([69, S], BF16, tag=f"kt{s}", name=f"KT{s}") for s in range(4)]
            Vb = sb.tile([P, NKT, 4, VW], BF16, tag="vb")
            OTs = [sb.tile([P, S], BF16, tag=f"ots{h}", name=f"OTs{h}") for h in range(2)]
            xts = [
                sb.tile([P, 8, QC], BF16, tag="xt", bufs=4, name=f"xt_{q}")
                for q in range(NQC)
            ]

            # x^T quarters on the sync queue (critical path), small tensors
            # on gpsimd so the triggers don't serialize behind them.
            for q in range(NQC):
                nc.sync.dma_start(
                    out=xts[q],
                    in_=xTd[:, QC * q: QC * (q + 1)].rearrange(
                        "(a p) q -> p a q", p=P
                    ),
                )
            nc.gpsimd.dma_start(out=wq_sb, in_=wq.rearrange("(a p) m -> p a m", p=P))
            nc.gpsimd.dma_start(out=wk_sb, in_=wk.rearrange("(a p) m -> p a m", p=P))
            nc.gpsimd.dma_start(out=wv_sb, in_=wv.rearrange("(a p) m -> p a m", p=P))
            nc.gpsimd.dma_start(out=wo_sb, in_=wo.rearrange("(h p) d -> p h d", p=P))
            nc.gpsimd.dma_start(out=mm_sb, in_=maskmin.rearrange("j p q -> p j q"))
            for s in range(4):
                nc.gpsimd.dma_start(out=QT[s][64:69, :], in_=qaug[s])
                nc.gpsimd.dma_start(out=KT[s][64:69, :], in_=kaug[s])
            # ones everywhere; V data columns overwritten below, leaving the
            # per-(tile,slot) ones column that yields the softmax denominator
            nc.vector.memset(Vb, 1.0)

            def proj_quarter(q):
                cs = slice(QC * q, QC * (q + 1))
                st = ps.tile([P, 4, QC], F32, tag="st", bufs=1, name=f"pj_{q}")
                for k in range(8):
                    for u, (w_sb, g) in enumerate(
                        ((wq_sb, 0), (wq_sb, 1), (wk_sb, 0), (wk_sb, 1))
                    ):
                        nc.tensor.matmul(
                            st[:, u, :],
                            w_sb[:, k, 128 * g: 128 * (g + 1)],
                            xts[q][:, k, :],
                            start=(k == 0), stop=(k == 7),
                        )
                for u, (T, se, so) in enumerate(
                    ((QT, 0, 1), (QT, 2, 3), (KT, 0, 1), (KT, 2, 3))
                ):
                    nc.vector.tensor_copy(out=T[se][0:64, cs], in_=st[0:64, u, :])
                    stg = sb.tile([P, QC], BF16, tag="stg", bufs=2,
                                  name=f"stg_{q}_{u}")
                    nc.scalar.copy(out=stg[64:128, :], in_=st[64:128, u, :])
                    nc.gpsimd.dma_start(out=T[so][0:64, cs], in_=stg[64:128, :])
                # V in natural [s, d] layout: stationary = x^T s-slice
                for j in range(4):
                    sti = 4 * q + j
                    vt = ps.tile([P, 256], F32, tag="ot", bufs=2, name=f"v_{sti}")
                    for k in range(8):
                        nc.tensor.matmul(
                            vt,
                            xts[q][:, k, P * j: P * (j + 1)],
                            wv_sb[:, k, :],
                            start=(k == 0), stop=(k == 7),
                        )
                    nc.vector.tensor_copy(
                        out=Vb[:, sti, :, 0:HD],
                        in_=vt.rearrange("p (s d) -> p s d", s=4),
                    )

            pending = []  # deferred O-projection emitters (PE fill work)

            def pop_fill():
                if pending:
                    pending.pop(0)()

            def oproj_stile(sti):
                def emit():
                    ost = sb.tile([P, D], BF16, tag="ost", bufs=3,
                                  name=f"os_{sti}")
                    for n in range(2):
                        op = ps.tile([P, QC], F32, tag="mm", bufs=2,
                                     name=f"op_{sti}_{n}")
                        for h in range(2):
                            nc.tensor.matmul(
                                op,
                                OTs[h][:, P * sti: P * (sti + 1)],
                                wo_sb[:, h, QC * n: QC * (n + 1)],
                                start=(h == 0), stop=(h == 1),
                            )
                        if n == 0:
                            nc.scalar.copy(out=ost[:, 0:QC], in_=op)
                        else:
                            nc.vector.tensor_copy(out=ost[:, QC:D], in_=op)
                    nc.sync.dma_start(
                        out=out[P * sti: P * (sti + 1), :], in_=ost
                    )
                return emit

            def attn_chunk(i):
                cs = slice(QC * i, QC * (i + 1))
                for a, b in ((0, 3), (1, 2)):
                    tl = {s: _tiles_for_chunk(i, s) for s in (a, b)}
                    otp = {
                        s: ps.tile([P, QC], F32, tag="ot", bufs=2,
                                   name=f"ot_{i}_{s}")
                        for s in (a, b)
                    }
                    ga, gb = _groups(tl[a]), _groups(tl[b])
                    rr = []
                    for z in range(max(len(ga), len(gb))):
                        if z < len(ga):
                            rr.append((a, ga[z]))
                        if z < len(gb):
                            rr.append((b, gb[z]))
                    for s, grp in rr:
                        n = len(grp)
                        st = ps.tile([P, 4, QC], F32, tag="st", bufs=1,
                                     name=f"st_{i}_{s}_{grp[0]}")
                        for u, t in enumerate(grp):
                            nc.tensor.matmul(
                                st[:, u, :],
                                KT[s][0:69, P * t: P * (t + 1)],
                                QT[s][0:69, cs],
                                start=True, stop=True,
                            )
                        pt = sb.tile([P, 4, QC], BF16, tag="pt", bufs=3,
                                     name=f"pt_{i}_{s}_{grp[0]}")
                        nc.scalar.activation(
                            out=pt[:, 0:n, :], in_=st[:, 0:n, :],
                            func=mybir.ActivationFunctionType.Exp,
                            bias=0.0, scale=1.0,
                        )
                        if grp[-1] == 4 * i + 3 and n == 4:
                            # diagonal quad: zero k>q via min with {1e9, 0}
                            nc.vector.tensor_tensor(
                                out=pt[:, 0:4, :], in0=pt[:, 0:4, :],
                                in1=mm_sb, op=mybir.AluOpType.min,
                            )
                        for u, t in enumerate(grp):
                            nc.tensor.matmul(
                                otp[s],
                                Vb[:, t, s, :],
                                pt[:, u, :],
                                start=(t == tl[s][0]), stop=(t == tl[s][-1]),
                            )
                        pop_fill()
                    for s in (a, b):
                        rec = sb.tile([P, QC], BF16, tag="rec", bufs=2,
                                      name=f"rec_{i}_{s}")
                        with nc.allow_low_precision("softmax 1/r in bf16"):
                            nc.vector.reciprocal(
                                out=rec[64:128, :], in_=otp[s][64:128, :]
                            )
                        rec2 = sb.tile([64, QC], BF16, tag="rec2", bufs=2,
                                       name=f"rec2_{i}_{s}")
                        nc.gpsimd.dma_start(out=rec2, in_=rec[64:128, :])
                        h, half = s // 2, s % 2
                        if half == 0:
                            nc.vector.tensor_tensor(
                                out=OTs[h][0:64, cs], in0=otp[s][0:64, :],
                                in1=rec2, op=mybir.AluOpType.mult,
                            )
                        else:
                            ott = sb.tile([64, QC], BF16, tag="ott", bufs=2,
                                          name=f"ott_{i}_{s}")
                            nc.vector.tensor_tensor(
                                out=ott, in0=otp[s][0:64, :], in1=rec2,
                                op=mybir.AluOpType.mult,
                            )
                            nc.gpsimd.dma_start(out=OTs[h][64:128, cs], in_=ott)
                for j in range(4):
                    pending.append(oproj_stile(4 * i + j))

            for q in range(NQC):
                proj_quarter(q)
            for i in range(NQC):
                attn_chunk(i)
            while pending:
                pending.pop(0)()
    nc.compile()
    return nc


_NC_CACHE = None


def _get_graph():
    global _NC_CACHE
    if _NC_CACHE is None:
        _NC_CACHE = build_graph()
    return _NC_CACHE


def _host_inputs(x, W_Q, W_K, W_V, W_O):
    """Per-core input maps."""
    x = np.asarray(x, dtype=np.float32)
    W_Q = np.asarray(W_Q, dtype=np.float32)
    W_K = np.asarray(W_K, dtype=np.float32)
    W_V = np.asarray(W_V, dtype=np.float32)
    W_O = np.asarray(W_O, dtype=np.float32)

    xT = ml_dtypes.bfloat16(np.ascontiguousarray(x.transpose(0, 2, 1)))  # [B,D,S]

    karr = np.arange(S, dtype=np.float64)
    kk = np.float32(karr % P)                  # 0..127, exact in bf16
    qh = np.float32((karr % QC) // 32)         # 0..15, exact
    ql = np.float32((karr % QC) % 32)          # 0..31, exact
    ones = np.ones(S, dtype=np.float32)

    mask = np.zeros((4, P, QC), dtype=np.float64)
    kki = np.arange(P)[:, None]
    qqi = np.arange(QC)[None, :]
    for j in range(4):
        mask[j] = np.where(kki + P * j <= qqi, 1e9, 0.0)
    maskmin = ml_dtypes.bfloat16(mask)

    in_maps = []
    for c in range(NCORES):
        b, j = divmod(c, 4)
        heads = [j, 4 + j, 8 + j, 12 + j]
        rows = np.concatenate(
            [np.arange(64 * h, 64 * h + 64) for h in heads]
        )
        wq = ml_dtypes.bfloat16(np.ascontiguousarray((W_Q[rows, :] / 8.0).T))
        wk = ml_dtypes.bfloat16(np.ascontiguousarray(W_K[rows, :].T))
        wv = ml_dtypes.bfloat16(np.ascontiguousarray(W_V[rows, :].T))
        wo = ml_dtypes.bfloat16(np.ascontiguousarray(W_O[:, rows].T))

        kaug = np.zeros((4, 5, S), dtype=np.float32)
        qaug = np.zeros((4, 5, S), dtype=np.float32)
        for s, h in enumerate(heads):
            sl = float(SLOPES[h])
            # sum of exact-in-bf16 products = sl * (k - q):
            #   kk*sl + (k-kk)*sl - 32sl*qh - sl*ql - sl*(q - q%512)
            kaug[s, 0] = kk
            kaug[s, 1] = np.float32(karr - (karr % P))
            kaug[s, 2] = np.float32(-32.0 * sl)
            kaug[s, 3] = np.float32(-sl)
            kaug[s, 4] = np.float32(-sl)
            qaug[s, 0] = np.float32(sl) * ones
            qaug[s, 1] = np.float32(sl) * ones
            qaug[s, 2] = qh
            qaug[s, 3] = ql
            qaug[s, 4] = np.float32(karr - (karr % QC))
        in_maps.append(
            {
                "xT": xT[b],
                "wq": wq,
                "wk": wk,
                "wv": wv,
                "wo": wo,
                "kaug": ml_dtypes.bfloat16(kaug),
                "qaug": ml_dtypes.bfloat16(qaug),
                "maskmin": maskmin,
            }
        )
    return in_maps


LAST_RESULTS = None


def kernel(x, W_Q, W_K, W_V, W_O):
    global LAST_RESULTS
    nc = _get_graph()
    in_maps = _host_inputs(x, W_Q, W_K, W_V, W_O)
    res = run_bass_kernel_spmd(nc, in_maps, core_ids=list(range(NCORES)))
    LAST_RESULTS = res
    total = np.zeros((B, S, D), dtype=np.float32)
    for c, r in enumerate(res.results):
        total[c // 4] += np.asarray(r["out"], dtype=np.float32)
    return total


if __name__ == "__main__":
    nc = build_graph()
    print("graph built ok")


# revision 6
# speedup vs baseline: 1.1982x; 1.1982x over previous
"""ALiBi causal attention (B=2, S=2048, D=1024, H=16) on 8 TRN2 NeuronCores.

Sharding: core c owns batch c//4 and heads {j, 4+j, 8+j, 12+j} (j = c%4),
one head per "slot" 0..3. All cores run the same graph (SPMD); slot s uses
the ALiBi window of its shallowest head (h = 4s+3), so per-slot k-tile
lists are core-independent.

Per core:
  - x^T streamed in 4 sequence-quarters; Q^T/K^T projected with quarter-wide
    matmuls (4 psum sub-slices of one 4-bank tile), V projected directly in
    natural [s, d] layout (stationary = x^T slice) - no PE transposes.
  - windowed causal flash-attention per slot, two slot-streams round-robin
    per q-chunk; ALiBi bias folded into the scores matmul via 5 exact
    bf16 "augmented" rows; exp on ScalarE in quad-tile ACTIVATEs
    ([128,4,512], amortizes the ~352-cycle fixed cost); diagonal quad
    masked post-exp by a min with {1e9,0}; P^T@V accumulated with a
    ones-column in V producing the softmax denominator; 1/rowsum via DVE
    reciprocal_approx_fast on the replicated rows, DMA-shifted to base 0.
  - W_O slice applied per 128-row tile, woven between attention groups of
    the next chunk to fill PE gaps while ScalarE runs exp.
Host sums the 4 partial outputs per batch (the W_O reduce) and reshapes.
"""

import math
import sys

sys.path.insert(0, "/opt/trn_rl_repo")

import ml_dtypes
import numpy as np

import concourse.bass as bass
import concourse.tile as tile
from concourse import bacc, mybir
from concourse.bass_utils import run_bass_kernel_spmd

B, S, D, H, HD = 2, 2048, 1024, 16, 64
P = 128          # k-tile / partition size
QC = 512         # q-chunk size
NKT = S // P     # 16 k-tiles
NQC = S // QC    # 4 q-chunks
NCORES = 8
T_WIN = 12.0     # ln cutoff: drop k-tiles with alibi penalty > T_WIN
VW = P           # V cols per (k-tile, slot): 64 data + 64 ones (the ones
                 # block makes P^T@V replicate the softmax denominator into
                 # psum rows 64:128, so no partition-broadcast is needed)

F32 = mybir.dt.float32
BF16 = mybir.dt.bfloat16

# slopes m_h = 2^-(h+1)/2, snapped to bf16 so every aug-row value is exactly
# representable regardless of PE input rounding.
SLOPES = np.float32(ml_dtypes.bfloat16(2.0 ** (-(np.arange(H, dtype=np.float64) + 1) / 2.0)))

# slot s covers heads 4s..4s+3 across the 4 cores of a batch; its k-tile
# window is set by the shallowest member (h = 4s+3). Slot 3 is full causal.
SLOT_L = [T_WIN / float(SLOPES[4 * s + 3]) for s in range(3)] + [None]


def _tiles_for_chunk(i: int, s: int):
    """k-tiles kept for q-chunk i of slot s (identical on all cores)."""
    if SLOT_L[s] is None:
        lo = 0
    else:
        lo = max(0, math.ceil((QC * i - SLOT_L[s] - (P - 1)) / P))
        if (4 * i + 4 - lo) % 2:  # keep counts even for pair/quad grouping
            lo = max(0, lo - 1)
    return list(range(lo, 4 * i + 4))


def _groups(lst):
    """Split an even-length ascending tile list into pairs."""
    out = []
    pos = 0
    while pos < len(lst):
        out.append(lst[pos:pos + 2])
        pos += 2
    return out


def _only_combined_act_table(arch):
    """Restrict the ACT LUT chooser so exp/copy resolve only to the one set
    holding them all - the Scalar engine never reloads tables mid-kernel."""
    from concourse import mybir as _mb
    from concourse.hw_specs import get_activation_tables as _orig
    tabs = _orig(arch)
    keep = "natural_log_exp_and_others"
    mine = {_mb.ActivationFunctionType.Exp, _mb.ActivationFunctionType.Ln,
            _mb.ActivationFunctionType.Copy, _mb.ActivationFunctionType.Identity}
    return {
        name: (fns if name == keep else (set(fns) - mine))
        for name, fns in tabs.items()
    }


def build_graph() -> bass.Bass:
    bacc.get_activation_tables = _only_combined_act_table
    nc = bacc.Bacc("TRN2", target_bir_lowering=False, debug=False)

    xTd = nc.dram_tensor("xT", (D, S), BF16, kind="ExternalInput").ap()
    wq = nc.dram_tensor("wq", (D, 256), BF16, kind="ExternalInput").ap()
    wk = nc.dram_tensor("wk", (D, 256), BF16, kind="ExternalInput").ap()
    wv = nc.dram_tensor("wv", (D, 256), BF16, kind="ExternalInput").ap()
    wo = nc.dram_tensor("wo", (256, D), BF16, kind="ExternalInput").ap()
    kaug = nc.dram_tensor("kaug", (4, 5, S), BF16, kind="ExternalInput").ap()
    qaug = nc.dram_tensor("qaug", (4, 5, S), BF16, kind="ExternalInput").ap()
    maskmin = nc.dram_tensor("maskmin", (4, P, QC), BF16, kind="ExternalInput").ap()
    out = nc.dram_tensor("out", (S, D), BF16, kind="ExternalOutput").ap()

    with tile.TileContext(nc) as tc:
        with (
            tc.tile_pool(name="sb", bufs=1) as sb,
            tc.tile_pool(name="ps", bufs=1, space="PSUM") as ps,
        ):
            # ---- persistent SBUF ----
            wq_sb = sb.tile([P, 8, 256], BF16, tag="wq")
            wk_sb = sb.tile([P, 8, 256], BF16, tag="wk")
            wv_sb = sb.tile([P, 8, 256], BF16, tag="wv")
            wo_sb = sb.tile([P, 2, D], BF16, tag="wo")
            mm_sb = sb.tile([P, 4, QC], BF16, tag="mask")
            QT = [sb.tile([69, S], BF16, tag=f"qt{s}", name=f"QT{s}") for s in range(4)]
            KT = [sb.tile([69, S], BF16, tag=f"kt{s}", name=f"KT{s}") for s in range(4)]
            Vb = sb.tile([P, NKT, 4, VW], BF16, tag="vb")
            OTs = [sb.tile([P, S], BF16, tag=f"ots{h}", name=f"OTs{h}") for h in range(2)]
            xts = [
                [
                    sb.tile([P, 4, QC], BF16, tag="xt", bufs=8,
                            name=f"xt_{q}_{h}")
                    for h in range(2)
                ]
                for q in range(NQC)
            ]

            # x^T in (quarter, dm-half) pieces, split across both DMA queues
            # for bandwidth; weights first on gpsimd (needed by the first MM).
            nc.gpsimd.dma_start(out=wq_sb, in_=wq.rearrange("(a p) m -> p a m", p=P))
            nc.gpsimd.dma_start(out=wk_sb, in_=wk.rearrange("(a p) m -> p a m", p=P))
            nc.gpsimd.dma_start(out=wv_sb, in_=wv.rearrange("(a p) m -> p a m", p=P))
            for q in range(NQC):
                for h, eng in ((0, nc.sync), (1, nc.gpsimd)):
                    eng.dma_start(
                        out=xts[q][h],
                        in_=xTd[
                            4 * P * h: 4 * P * (h + 1), QC * q: QC * (q + 1)
                        ].rearrange("(a p) q -> p a q", p=P),
                    )
            nc.gpsimd.dma_start(out=wo_sb, in_=wo.rearrange("(h p) d -> p h d", p=P))
            nc.gpsimd.dma_start(out=mm_sb, in_=maskmin.rearrange("j p q -> p j q"))
            for s in range(4):
                nc.gpsimd.dma_start(out=QT[s][64:69, :], in_=qaug[s])
                nc.gpsimd.dma_start(out=KT[s][64:69, :], in_=kaug[s])
            # ones everywhere; V data columns overwritten below, leaving the
            # per-(tile,slot) ones column that yields the softmax denominator
            nc.vector.memset(Vb, 1.0)

            def proj_quarter(q):
                cs = slice(QC * q, QC * (q + 1))
                stq = ps.tile([P, 2, QC], F32, tag="st", bufs=3, name=f"pjq_{q}")
                stk = ps.tile([P, 2, QC], F32, tag="st", bufs=3, name=f"pjk_{q}")
                for k in range(8):
                    xsl = xts[q][k // 4][:, k % 4, :]
                    for st, w_sb in ((stq, wq_sb), (stk, wk_sb)):
                        for g in range(2):
                            nc.tensor.matmul(
                                st[:, g, :],
                                w_sb[:, k, 128 * g: 128 * (g + 1)],
                                xsl,
                                start=(k == 0), stop=(k == 7),
                            )
                for st, T in ((stq, QT), (stk, KT)):
                    for g in range(2):
                        nc.vector.tensor_copy(
                            out=T[2 * g][0:64, cs], in_=st[0:64, g, :]
                        )
                        stg = sb.tile([P, QC], BF16, tag="stg", bufs=2,
                                      name=f"stg_{q}_{id(T)}_{g}")
                        nc.scalar.copy(out=stg[64:128, :], in_=st[64:128, g, :])
                        nc.gpsimd.dma_start(
                            out=T[2 * g + 1][0:64, cs], in_=stg[64:128, :]
                        )
                # V in natural [s, d] layout: stationary = x^T s-slice
                for j in range(4):
                    sti = 4 * q + j
                    vt = ps.tile([P, 256], F32, tag="ot", bufs=2, name=f"v_{sti}")
                    for k in range(8):
                        nc.tensor.matmul(
                            vt,
                            xts[q][k // 4][:, k % 4, P * j: P * (j + 1)],
                            wv_sb[:, k, :],
                            start=(k == 0), stop=(k == 7),
                        )
                    nc.vector.tensor_copy(
                        out=Vb[:, sti, :, 0:HD],
                        in_=vt.rearrange("p (s d) -> p s d", s=4),
                    )

            pending = []  # deferred O-projection emitters (PE fill work)

            def pop_fill():
                if pending:
                    pending.pop(0)()

            def oproj_half(sti, n, ost):
                def emit():
                    op = ps.tile([P, QC], F32, tag="st", bufs=3,
                                 name=f"op_{sti}_{n}")
                    for h in range(2):
                        nc.tensor.matmul(
                            op,
                            OTs[h][:, P * sti: P * (sti + 1)],
                            wo_sb[:, h, QC * n: QC * (n + 1)],
                            start=(h == 0), stop=(h == 1),
                        )
                    if n == 0:
                        nc.scalar.copy(out=ost[:, 0:QC], in_=op)
                    else:
                        nc.vector.tensor_copy(out=ost[:, QC:D], in_=op)
                        nc.sync.dma_start(
                            out=out[P * sti: P * (sti + 1), :], in_=ost
                        )
                return emit

            def attn_chunk(i):
                cs = slice(QC * i, QC * (i + 1))
                for a, b in ((0, 3), (1, 2)):
                    tl = {s: _tiles_for_chunk(i, s) for s in (a, b)}
                    otp = {
                        s: ps.tile([P, QC], F32, tag="ot", bufs=2,
                                   name=f"ot_{i}_{s}")
                        for s in (a, b)
                    }
                    def finalize(s):
                        rec = sb.tile([P, QC], F32, tag="rec", bufs=2,
                                      name=f"rec_{i}_{s}")
                        with nc.allow_low_precision("softmax 1/r"):
                            nc.vector.reciprocal(
                                out=rec[64:128, :], in_=otp[s][64:128, :]
                            )
                        rec2 = sb.tile([64, QC], F32, tag="rec2", bufs=2,
                                       name=f"rec2_{i}_{s}")
                        nc.gpsimd.dma_start(out=rec2, in_=rec[64:128, :])
                        h, half = s // 2, s % 2
                        if half == 0:
                            nc.vector.tensor_tensor(
                                out=OTs[h][0:64, cs], in0=otp[s][0:64, :],
                                in1=rec2, op=mybir.AluOpType.mult,
                            )
                        else:
                            ott = sb.tile([64, QC], BF16, tag="ott", bufs=2,
                                          name=f"ott_{i}_{s}")
                            nc.vector.tensor_tensor(
                                out=ott, in0=otp[s][0:64, :], in1=rec2,
                                op=mybir.AluOpType.mult,
                            )
                            nc.gpsimd.dma_start(out=OTs[h][64:128, cs], in_=ott)

                    ga, gb = _groups(tl[a]), _groups(tl[b])
                    rr = []
                    for z in range(max(len(ga), len(gb))):
                        if z < len(ga):
                            rr.append((a, ga[z], z == len(ga) - 1))
                        if z < len(gb):
                            rr.append((b, gb[z], z == len(gb) - 1))
                    for s, grp, last in rr:
                        st = ps.tile([P, 2, QC], F32, tag="st", bufs=3,
                                     name=f"st_{i}_{s}_{grp[0]}")
                        for u, t in enumerate(grp):
                            nc.tensor.matmul(
                                st[:, u, :],
                                KT[s][0:69, P * t: P * (t + 1)],
                                QT[s][0:69, cs],
                                start=True, stop=True,
                            )
                        pt = sb.tile([P, 2, QC], BF16, tag="pt", bufs=4,
                                     name=f"pt_{i}_{s}_{grp[0]}")
                        nc.scalar.activation(
                            out=pt, in_=st,
                            func=mybir.ActivationFunctionType.Exp,
                            bias=0.0, scale=1.0,
                        )
                        j = grp[0] - 4 * i
                        if j >= 0:
                            # diagonal pair: zero k>q via min with {1e9, 0}
                            nc.vector.tensor_tensor(
                                out=pt, in0=pt, in1=mm_sb[:, j: j + 2, :],
                                op=mybir.AluOpType.min,
                            )
                        for u, t in enumerate(grp):
                            nc.tensor.matmul(
                                otp[s],
                                Vb[:, t, s, :],
                                pt[:, u, :],
                                start=(t == tl[s][0]), stop=(t == tl[s][-1]),
                            )
                        pop_fill()
                        if last:
                            finalize(s)
                for j in range(4):
                    sti = 4 * i + j
                    ost = sb.tile([P, D], BF16, tag="ost", bufs=3,
                                  name=f"os_{sti}")
                    pending.append(oproj_half(sti, 0, ost))
                    pending.append(oproj_half(sti, 1, ost))

            for q in range(NQC):
                proj_quarter(q)
            for i in range(NQC):
                attn_chunk(i)
            while pending:
                pending.pop(0)()
    nc.compile()
    return nc


_NC_CACHE = None


def _get_graph():
    global _NC_CACHE
    if _NC_CACHE is None:
        _NC_CACHE = build_graph()
    return _NC_CACHE


def _host_inputs(x, W_Q, W_K, W_V, W_O):
    """Per-core input maps."""
    x = np.asarray(x, dtype=np.float32)
    W_Q = np.asarray(W_Q, dtype=np.float32)
    W_K = np.asarray(W_K, dtype=np.float32)
    W_V = np.asarray(W_V, dtype=np.float32)
    W_O = np.asarray(W_O, dtype=np.float32)

    xT = ml_dtypes.bfloat16(np.ascontiguousarray(x.transpose(0, 2, 1)))  # [B,D,S]

    karr = np.arange(S, dtype=np.float64)
    kk = np.float32(karr % P)                  # 0..127, exact in bf16
    qh = np.float32((karr % QC) // 32)         # 0..15, exact
    ql = np.float32((karr % QC) % 32)          # 0..31, exact
    ones = np.ones(S, dtype=np.float32)

    mask = np.zeros((4, P, QC), dtype=np.float64)
    kki = np.arange(P)[:, None]
    qqi = np.arange(QC)[None, :]
    for j in range(4):
        mask[j] = np.where(kki + P * j <= qqi, 1e9, 0.0)
    maskmin = ml_dtypes.bfloat16(mask)

    in_maps = []
    for c in range(NCORES):
        b, j = divmod(c, 4)
        heads = [j, 4 + j, 8 + j, 12 + j]
        rows = np.concatenate(
            [np.arange(64 * h, 64 * h + 64) for h in heads]
        )
        wq = ml_dtypes.bfloat16(np.ascontiguousarray((W_Q[rows, :] / 8.0).T))
        wk = ml_dtypes.bfloat16(np.ascontiguousarray(W_K[rows, :].T))
        wv = ml_dtypes.bfloat16(np.ascontiguousarray(W_V[rows, :].T))
        wo = ml_dtypes.bfloat16(np.ascontiguousarray(W_O[:, rows].T))

        kaug = np.zeros((4, 5, S), dtype=np.float32)
        qaug = np.zeros((4, 5, S), dtype=np.float32)
        for s, h in enumerate(heads):
            sl = float(SLOPES[h])
            # sum of exact-in-bf16 products = sl * (k - q):
            #   kk*sl + (k-kk)*sl - 32sl*qh - sl*ql - sl*(q - q%512)
            kaug[s, 0] = kk
            kaug[s, 1] = np.float32(karr - (karr % P))
            kaug[s, 2] = np.float32(-32.0 * sl)
            kaug[s, 3] = np.float32(-sl)
            kaug[s, 4] = np.float32(-sl)
            qaug[s, 0] = np.float32(sl) * ones
            qaug[s, 1] = np.float32(sl) * ones
            qaug[s, 2] = qh
            qaug[s, 3] = ql
            qaug[s, 4] = np.float32(karr - (karr % QC))
        in_maps.append(
            {
                "xT": xT[b],
                "wq": wq,
                "wk": wk,
                "wv": wv,
                "wo": wo,
                "kaug": ml_dtypes.bfloat16(kaug),
                "qaug": ml_dtypes.bfloat16(qaug),
                "maskmin": maskmin,
            }
        )
    return in_maps


LAST_RESULTS = None


def kernel(x, W_Q, W_K, W_V, W_O):
    global LAST_RESULTS
    nc = _get_graph()
    in_maps = _host_inputs(x, W_Q, W_K, W_V, W_O)
    res = run_bass_kernel_spmd(nc, in_maps, core_ids=list(range(NCORES)))
    LAST_RESULTS = res
    total = np.zeros((B, S, D), dtype=np.float32)
    for c, r in enumerate(res.results):
        total[c // 4] += np.asarray(r["out"], dtype=np.float32)
    return total


if __name__ == "__main__":
    nc = build_graph()
    print("graph built ok")


# revision 9
# speedup vs baseline: 1.2066x; 1.0070x over previous
"""ALiBi causal attention (B=2, S=2048, D=1024, H=16) on 8 TRN2 NeuronCores.

Sharding: core c owns batch c//4 and heads {j, 4+j, 8+j, 12+j} (j = c%4),
one head per "slot" 0..3. All cores run the same graph (SPMD); slot s uses
the ALiBi window of its shallowest head (h = 4s+3), so per-slot k-tile
lists are core-independent.

Per core:
  - x^T streamed in 4 sequence-quarters; Q^T/K^T projected with quarter-wide
    matmuls (4 psum sub-slices of one 4-bank tile), V projected directly in
    natural [s, d] layout (stationary = x^T slice) - no PE transposes.
  - windowed causal flash-attention per slot, two slot-streams round-robin
    per q-chunk; ALiBi bias folded into the scores matmul via 5 exact
    bf16 "augmented" rows; exp on ScalarE in quad-tile ACTIVATEs
    ([128,4,512], amortizes the ~352-cycle fixed cost); diagonal quad
    masked post-exp by a min with {1e9,0}; P^T@V accumulated with a
    ones-column in V producing the softmax denominator; 1/rowsum via DVE
    reciprocal_approx_fast on the replicated rows, DMA-shifted to base 0.
  - W_O slice applied per 128-row tile, woven between attention groups of
    the next chunk to fill PE gaps while ScalarE runs exp.
Host sums the 4 partial outputs per batch (the W_O reduce) and reshapes.
"""

import math
import sys

sys.path.insert(0, "/opt/trn_rl_repo")

import ml_dtypes
import numpy as np

import concourse.bass as bass
import concourse.tile as tile
from concourse import bacc, mybir
from concourse.bass_utils import run_bass_kernel_spmd

B, S, D, H, HD = 2, 2048, 1024, 16, 64
P = 128          # k-tile / partition size
QC = 512         # q-chunk size
NKT = S // P     # 16 k-tiles
NQC = S // QC    # 4 q-chunks
NCORES = 8
T_WIN = 12.0     # ln cutoff: drop k-tiles with alibi penalty > T_WIN
VW = P           # V cols per (k-tile, slot): 64 data + 64 ones (the ones
                 # block makes P^T@V replicate the softmax denominator into
                 # psum rows 64:128, so no partition-broadcast is needed)

F32 = mybir.dt.float32
BF16 = mybir.dt.bfloat16

# slopes m_h = 2^-(h+1)/2, snapped to bf16 so every aug-row value is exactly
# representable regardless of PE input rounding.
SLOPES = np.float32(ml_dtypes.bfloat16(2.0 ** (-(np.arange(H, dtype=np.float64) + 1) / 2.0)))

# slot s covers heads 4s..4s+3 across the 4 cores of a batch; its k-tile
# window is set by the shallowest member (h = 4s+3). Slot 3 is full causal.
SLOT_L = [T_WIN / float(SLOPES[4 * s + 3]) for s in range(3)] + [None]


def _tiles_for_chunk(i: int, s: int):
    """k-tiles kept for q-chunk i of slot s (identical on all cores)."""
    if SLOT_L[s] is None:
        lo = 0
    else:
        lo = max(0, math.ceil((QC * i - SLOT_L[s] - (P - 1)) / P))
        if (4 * i + 4 - lo) % 2:  # keep counts even for pair/quad grouping
            lo = max(0, lo - 1)
    return list(range(lo, 4 * i + 4))


def _groups(lst):
    """Split an even-length ascending tile list into pairs."""
    out = []
    pos = 0
    while pos < len(lst):
        out.append(lst[pos:pos + 2])
        pos += 2
    return out


def _only_combined_act_table(arch):
    """Restrict the ACT LUT chooser so exp/copy resolve only to the one set
    holding them all - the Scalar engine never reloads tables mid-kernel."""
    from concourse import mybir as _mb
    from concourse.hw_specs import get_activation_tables as _orig
    tabs = _orig(arch)
    keep = "natural_log_exp_and_others"
    mine = {_mb.ActivationFunctionType.Exp, _mb.ActivationFunctionType.Ln,
            _mb.ActivationFunctionType.Copy, _mb.ActivationFunctionType.Identity}
    return {
        name: (fns if name == keep else (set(fns) - mine))
        for name, fns in tabs.items()
    }


def build_graph() -> bass.Bass:
    bacc.get_activation_tables = _only_combined_act_table
    nc = bacc.Bacc("TRN2", target_bir_lowering=False, debug=False)

    xTd = nc.dram_tensor("xT", (NQC, 2, P, 4, QC), BF16, kind="ExternalInput").ap()
    wq = nc.dram_tensor("wq", (D, 256), BF16, kind="ExternalInput").ap()
    wk = nc.dram_tensor("wk", (D, 256), BF16, kind="ExternalInput").ap()
    wv = nc.dram_tensor("wv", (D, 256), BF16, kind="ExternalInput").ap()
    wo = nc.dram_tensor("wo", (256, D), BF16, kind="ExternalInput").ap()
    kaug = nc.dram_tensor("kaug", (4, 5, S), BF16, kind="ExternalInput").ap()
    qaug = nc.dram_tensor("qaug", (4, 5, S), BF16, kind="ExternalInput").ap()
    maskmin = nc.dram_tensor("maskmin", (4, P, QC), BF16, kind="ExternalInput").ap()
    out = nc.dram_tensor("out", (S, D), BF16, kind="ExternalOutput").ap()

    with tile.TileContext(nc) as tc:
        with (
            tc.tile_pool(name="sb", bufs=1) as sb,
            tc.tile_pool(name="ps", bufs=1, space="PSUM") as ps,
        ):
            # ---- persistent SBUF ----
            wq_sb = sb.tile([P, 8, 256], BF16, tag="wq")
            wk_sb = sb.tile([P, 8, 256], BF16, tag="wk")
            wv_sb = sb.tile([P, 8, 256], BF16, tag="wv")
            wo_sb = sb.tile([P, 2, D], BF16, tag="wo")
            mm_sb = sb.tile([P, 4, QC], BF16, tag="mask")
            QT = [sb.tile([69, S], BF16, tag=f"qt{s}", name=f"QT{s}") for s in range(4)]
            KT = [sb.tile([69, S], BF16, tag=f"kt{s}", name=f"KT{s}") for s in range(4)]
            Vb = sb.tile([P, NKT, 4, VW], BF16, tag="vb")
            OTs = [sb.tile([P, S], BF16, tag=f"ots{h}", name=f"OTs{h}") for h in range(2)]
            xts = [
                [
                    sb.tile([P, 4, QC], BF16, tag="xt", bufs=8,
                            name=f"xt_{q}_{h}")
                    for h in range(2)
                ]
                for q in range(NQC)
            ]

            # x^T in (quarter, dm-half) pieces, split across both DMA queues
            # for bandwidth; weights first on gpsimd (needed by the first MM).
            nc.gpsimd.dma_start(out=wq_sb, in_=wq.rearrange("(a p) m -> p a m", p=P))
            nc.gpsimd.dma_start(out=wk_sb, in_=wk.rearrange("(a p) m -> p a m", p=P))
            nc.gpsimd.dma_start(out=wv_sb, in_=wv.rearrange("(a p) m -> p a m", p=P))
            for q in range(NQC):
                for h, eng in ((0, nc.sync), (1, nc.gpsimd)):
                    eng.dma_start(out=xts[q][h], in_=xTd[q, h])
            nc.gpsimd.dma_start(out=wo_sb, in_=wo.rearrange("(h p) d -> p h d", p=P))
            nc.gpsimd.dma_start(out=mm_sb, in_=maskmin.rearrange("j p q -> p j q"))
            for s in range(4):
                nc.gpsimd.dma_start(out=QT[s][64:69, :], in_=qaug[s])
                nc.gpsimd.dma_start(out=KT[s][64:69, :], in_=kaug[s])
            # ones everywhere; V data columns overwritten below, leaving the
            # per-(tile,slot) ones column that yields the softmax denominator
            nc.vector.memset(Vb, 1.0)

            def proj_quarter(q):
                cs = slice(QC * q, QC * (q + 1))
                stq = ps.tile([P, 2, QC], F32, tag="st", bufs=2, name=f"pjq_{q}")
                stk = ps.tile([P, 2, QC], F32, tag="st", bufs=2, name=f"pjk_{q}")
                for k in range(8):
                    xsl = xts[q][k // 4][:, k % 4, :]
                    for st, w_sb in ((stq, wq_sb), (stk, wk_sb)):
                        for g in range(2):
                            nc.tensor.matmul(
                                st[:, g, :],
                                w_sb[:, k, 128 * g: 128 * (g + 1)],
                                xsl,
                                start=(k == 0), stop=(k == 7),
                            )
                for st, T in ((stq, QT), (stk, KT)):
                    for g in range(2):
                        nc.vector.tensor_copy(
                            out=T[2 * g][0:64, cs], in_=st[0:64, g, :]
                        )
                        stg = sb.tile([P, QC], BF16, tag="stg", bufs=2,
                                      name=f"stg_{q}_{id(T)}_{g}")
                        nc.scalar.copy(out=stg[64:128, :], in_=st[64:128, g, :])
                        nc.gpsimd.dma_start(
                            out=T[2 * g + 1][0:64, cs], in_=stg[64:128, :]
                        )
                # V in natural [s, d] layout: stationary = x^T s-slice
                for j in range(4):
                    sti = 4 * q + j
                    vt = ps.tile([P, 256], F32, tag="ot", bufs=4, name=f"v_{sti}")
                    for k in range(8):
                        nc.tensor.matmul(
                            vt,
                            xts[q][k // 4][:, k % 4, P * j: P * (j + 1)],
                            wv_sb[:, k, :],
                            start=(k == 0), stop=(k == 7),
                        )
                    nc.vector.tensor_copy(
                        out=Vb[:, sti, :, 0:HD],
                        in_=vt.rearrange("p (s d) -> p s d", s=4),
                    )

            pending = []  # deferred O-projection emitters (PE fill work)

            def pop_fill():
                if pending:
                    pending.pop(0)()

            def oproj_half(sti, n, ost):
                def emit():
                    op = ps.tile([P, QC], F32, tag="st", bufs=2,
                                 name=f"op_{sti}_{n}")
                    for h in range(2):
                        nc.tensor.matmul(
                            op,
                            OTs[h][:, P * sti: P * (sti + 1)],
                            wo_sb[:, h, QC * n: QC * (n + 1)],
                            start=(h == 0), stop=(h == 1),
                        )
                    if n == 0:
                        nc.scalar.copy(out=ost[:, 0:QC], in_=op)
                    else:
                        nc.vector.tensor_copy(out=ost[:, QC:D], in_=op)
                        nc.sync.dma_start(
                            out=out[P * sti: P * (sti + 1), :], in_=ost
                        )
                return emit

            def attn_chunk(i):
                cs = slice(QC * i, QC * (i + 1))
                for a, b in ((0, 3), (1, 2)):
                    tl = {s: _tiles_for_chunk(i, s) for s in (a, b)}
                    otp = {
                        s: ps.tile([P, QC], F32, tag="ot", bufs=4,
                                   name=f"ot_{i}_{s}")
                        for s in (a, b)
                    }
                    def finalize(s):
                        recf = sb.tile([P, QC], F32, tag="recf", bufs=2,
                                       name=f"recf_{i}_{s}")
                        rec = sb.tile([P, QC], BF16, tag="rec", bufs=2,
                                      name=f"rec_{i}_{s}")
                        nc.scalar.activation(
                            out=recf[64:128, :], in_=otp[s][64:128, :],
                            func=mybir.ActivationFunctionType.Ln,
                        )
                        nc.scalar.activation(
                            out=rec[64:128, :], in_=recf[64:128, :],
                            func=mybir.ActivationFunctionType.Exp, scale=-1.0,
                        )
                        rec2 = sb.tile([64, QC], BF16, tag="rec2", bufs=2,
                                       name=f"rec2_{i}_{s}")
                        nc.gpsimd.dma_start(out=rec2, in_=rec[64:128, :])
                        h, half = s // 2, s % 2
                        if half == 0:
                            nc.vector.tensor_tensor(
                                out=OTs[h][0:64, cs], in0=otp[s][0:64, :],
                                in1=rec2, op=mybir.AluOpType.mult,
                            )
                        else:
                            ott = sb.tile([64, QC], BF16, tag="ott", bufs=2,
                                          name=f"ott_{i}_{s}")
                            nc.vector.tensor_tensor(
                                out=ott, in0=otp[s][0:64, :], in1=rec2,
                                op=mybir.AluOpType.mult,
                            )
                            nc.gpsimd.dma_start(out=OTs[h][64:128, cs], in_=ott)

                    ga, gb = _groups(tl[a]), _groups(tl[b])
                    rr = []
                    for z in range(max(len(ga), len(gb))):
                        if z < len(ga):
                            rr.append((a, ga[z], z == len(ga) - 1))
                        if z < len(gb):
                            rr.append((b, gb[z], z == len(gb) - 1))
                    for s, grp, last in rr:
                        st = ps.tile([P, 2, QC], F32, tag="st", bufs=2,
                                     name=f"st_{i}_{s}_{grp[0]}")
                        for u, t in enumerate(grp):
                            nc.tensor.matmul(
                                st[:, u, :],
                                KT[s][0:69, P * t: P * (t + 1)],
                                QT[s][0:69, cs],
                                start=True, stop=True,
                            )
                        pt = sb.tile([P, 2, QC], BF16, tag="pt", bufs=6,
                                     name=f"pt_{i}_{s}_{grp[0]}")
                        nc.scalar.activation(
                            out=pt, in_=st,
                            func=mybir.ActivationFunctionType.Exp,
                            bias=0.0, scale=1.0,
                        )
                        j = grp[0] - 4 * i
                        if j >= 0:
                            # diagonal pair: zero k>q via min with {1e9, 0}
                            nc.vector.tensor_tensor(
                                out=pt, in0=pt, in1=mm_sb[:, j: j + 2, :],
                                op=mybir.AluOpType.min,
                            )
                        for u, t in enumerate(grp):
                            nc.tensor.matmul(
                                otp[s],
                                Vb[:, t, s, :],
                                pt[:, u, :],
                                start=(t == tl[s][0]), stop=(t == tl[s][-1]),
                            )
                        pop_fill()
                        if last:
                            finalize(s)
                for j in range(4):
                    sti = 4 * i + j
                    ost = sb.tile([P, D], BF16, tag="ost", bufs=3,
                                  name=f"os_{sti}")
                    pending.append(oproj_half(sti, 0, ost))
                    pending.append(oproj_half(sti, 1, ost))

            for q in range(NQC):
                proj_quarter(q)
            for i in range(NQC):
                attn_chunk(i)
            while pending:
                pending.pop(0)()
    nc.compile()
    return nc


_NC_CACHE = None


def _get_graph():
    global _NC_CACHE
    if _NC_CACHE is None:
        _NC_CACHE = build_graph()
    return _NC_CACHE


def _host_inputs(x, W_Q, W_K, W_V, W_O):
    """Per-core input maps."""
    x = np.asarray(x, dtype=np.float32)
    W_Q = np.asarray(W_Q, dtype=np.float32)
    W_K = np.asarray(W_K, dtype=np.float32)
    W_V = np.asarray(W_V, dtype=np.float32)
    W_O = np.asarray(W_O, dtype=np.float32)

    xT = ml_dtypes.bfloat16(np.ascontiguousarray(x.transpose(0, 2, 1)))  # [B,D,S]
    # pre-tile to [NQC, dm-half, partition, dm-block, q] so each (quarter,
    # half) DMA reads contiguous 4KB lines per partition
    xTq = np.ascontiguousarray(
        xT.reshape(B, 2, 4, P, NQC, QC).transpose(0, 4, 1, 3, 2, 5)
    )

    karr = np.arange(S, dtype=np.float64)
    kk = np.float32(karr % P)                  # 0..127, exact in bf16
    qh = np.float32((karr % QC) // 32)         # 0..15, exact
    ql = np.float32((karr % QC) % 32)          # 0..31, exact
    ones = np.ones(S, dtype=np.float32)

    mask = np.zeros((4, P, QC), dtype=np.float64)
    kki = np.arange(P)[:, None]
    qqi = np.arange(QC)[None, :]
    for j in range(4):
        mask[j] = np.where(kki + P * j <= qqi, 1e9, 0.0)
    maskmin = ml_dtypes.bfloat16(mask)

    in_maps = []
    for c in range(NCORES):
        b, j = divmod(c, 4)
        heads = [j, 4 + j, 8 + j, 12 + j]
        rows = np.concatenate(
            [np.arange(64 * h, 64 * h + 64) for h in heads]
        )
        wq = ml_dtypes.bfloat16(np.ascontiguousarray((W_Q[rows, :] / 8.0).T))
        wk = ml_dtypes.bfloat16(np.ascontiguousarray(W_K[rows, :].T))
        wv = ml_dtypes.bfloat16(np.ascontiguousarray(W_V[rows, :].T))
        wo = ml_dtypes.bfloat16(np.ascontiguousarray(W_O[:, rows].T))

        kaug = np.zeros((4, 5, S), dtype=np.float32)
        qaug = np.zeros((4, 5, S), dtype=np.float32)
        for s, h in enumerate(heads):
            sl = float(SLOPES[h])
            # sum of exact-in-bf16 products = sl * (k - q):
            #   kk*sl + (k-kk)*sl - 32sl*qh - sl*ql - sl*(q - q%512)
            kaug[s, 0] = kk
            kaug[s, 1] = np.float32(karr - (karr % P))
            kaug[s, 2] = np.float32(-32.0 * sl)
            kaug[s, 3] = np.float32(-sl)
            kaug[s, 4] = np.float32(-sl)
            qaug[s, 0] = np.float32(sl) * ones
            qaug[s, 1] = np.float32(sl) * ones
            qaug[s, 2] = qh
            qaug[s, 3] = ql
            qaug[s, 4] = np.float32(karr - (karr % QC))
        in_maps.append(
            {
                "xT": xTq[b],
                "wq": wq,
                "wk": wk,
                "wv": wv,
                "wo": wo,
                "kaug": ml_dtypes.bfloat16(kaug),
                "qaug": ml_dtypes.bfloat16(qaug),
                "maskmin": maskmin,
            }
        )
    return in_maps


LAST_RESULTS = None


def kernel(x, W_Q, W_K, W_V, W_O):
    global LAST_RESULTS
    nc = _get_graph()
    in_maps = _host_inputs(x, W_Q, W_K, W_V, W_O)
    res = run_bass_kernel_spmd(nc, in_maps, core_ids=list(range(NCORES)))
    LAST_RESULTS = res
    total = np.zeros((B, S, D), dtype=np.float32)
    for c, r in enumerate(res.results):
        total[c // 4] += np.asarray(r["out"], dtype=np.float32)
    return total


if __name__ == "__main__":
    nc = build_graph()
    print("graph built ok")


# revision 11
# speedup vs baseline: 1.2500x; 1.0359x over previous
"""ALiBi causal attention (B=2, S=2048, D=1024, H=16) on 8 TRN2 NeuronCores.

Sharding: core c owns batch c//4 and heads {j, 4+j, 8+j, 12+j} (j = c%4),
one head per "slot" 0..3. All cores run the same graph (SPMD); slot s uses
the ALiBi window of its shallowest head (h = 4s+3), so per-slot k-tile
lists are core-independent.

Per core:
  - x^T streamed in 4 sequence-quarters; Q^T/K^T projected with quarter-wide
    matmuls (4 psum sub-slices of one 4-bank tile), V projected directly in
    natural [s, d] layout (stationary = x^T slice) - no PE transposes.
  - windowed causal flash-attention per slot, two slot-streams round-robin
    per q-chunk; ALiBi bias folded into the scores matmul via 5 exact
    bf16 "augmented" rows; exp on ScalarE in quad-tile ACTIVATEs
    ([128,4,512], amortizes the ~352-cycle fixed cost); diagonal quad
    masked post-exp by a min with {1e9,0}; P^T@V accumulated with a
    ones-column in V producing the softmax denominator; 1/rowsum via DVE
    reciprocal_approx_fast on the replicated rows, DMA-shifted to base 0.
  - W_O slice applied per 128-row tile, woven between attention groups of
    the next chunk to fill PE gaps while ScalarE runs exp.
Host sums the 4 partial outputs per batch (the W_O reduce) and reshapes.
"""

import math
import sys

sys.path.insert(0, "/opt/trn_rl_repo")

import ml_dtypes
import numpy as np

import concourse.bass as bass
import concourse.tile as tile
from concourse import bacc, mybir
from concourse.bass_utils import run_bass_kernel_spmd

B, S, D, H, HD = 2, 2048, 1024, 16, 64
P = 128          # k-tile / partition size
QC = 512         # q-chunk size
NKT = S // P     # 16 k-tiles
NQC = S // QC    # 4 q-chunks
NCORES = 8
T_WIN = 12.0     # ln cutoff: drop k-tiles with alibi penalty > T_WIN
VW = P           # V cols per (k-tile, slot): 64 data + 64 ones (the ones
                 # block makes P^T@V replicate the softmax denominator into
                 # psum rows 64:128, so no partition-broadcast is needed)

F32 = mybir.dt.float32
BF16 = mybir.dt.bfloat16

# slopes m_h = 2^-(h+1)/2, snapped to bf16 so every aug-row value is exactly
# representable regardless of PE input rounding.
SLOPES = np.float32(ml_dtypes.bfloat16(2.0 ** (-(np.arange(H, dtype=np.float64) + 1) / 2.0)))

# slot s covers heads 4s..4s+3 across the 4 cores of a batch; its k-tile
# window is set by the shallowest member (h = 4s+3). Slot 3 is full causal.
SLOT_L = [T_WIN / float(SLOPES[4 * s + 3]) for s in range(3)] + [None]


def _tiles_for_chunk(i: int, s: int):
    """k-tiles kept for q-chunk i of slot s (identical on all cores)."""
    if SLOT_L[s] is None:
        lo = 0
    else:
        lo = max(0, math.ceil((QC * i - SLOT_L[s] - (P - 1)) / P))
        if (4 * i + 4 - lo) % 2:  # keep counts even for pair/quad grouping
            lo = max(0, lo - 1)
    return list(range(lo, 4 * i + 4))


def _groups(lst):
    """Split an even-length ascending tile list into pairs."""
    out = []
    pos = 0
    while pos < len(lst):
        out.append(lst[pos:pos + 2])
        pos += 2
    return out


def _only_combined_act_table(arch):
    """Restrict the ACT LUT chooser so exp/copy resolve only to the one set
    holding them all - the Scalar engine never reloads tables mid-kernel."""
    from concourse import mybir as _mb
    from concourse.hw_specs import get_activation_tables as _orig
    tabs = _orig(arch)
    keep = "natural_log_exp_and_others"
    mine = {_mb.ActivationFunctionType.Exp, _mb.ActivationFunctionType.Ln,
            _mb.ActivationFunctionType.Copy, _mb.ActivationFunctionType.Identity}
    return {
        name: (fns if name == keep else (set(fns) - mine))
        for name, fns in tabs.items()
    }


def build_graph() -> bass.Bass:
    bacc.get_activation_tables = _only_combined_act_table
    nc = bacc.Bacc("TRN2", target_bir_lowering=False, debug=False)

    xTd = nc.dram_tensor("xT", (NQC, 2, P, 4, QC), BF16, kind="ExternalInput").ap()
    wq = nc.dram_tensor("wq", (D, 256), BF16, kind="ExternalInput").ap()
    wk = nc.dram_tensor("wk", (D, 256), BF16, kind="ExternalInput").ap()
    wv = nc.dram_tensor("wv", (D, 256), BF16, kind="ExternalInput").ap()
    wo = nc.dram_tensor("wo", (256, D), BF16, kind="ExternalInput").ap()
    kaug = nc.dram_tensor("kaug", (4, 5, S), BF16, kind="ExternalInput").ap()
    qaug = nc.dram_tensor("qaug", (4, 5, S), BF16, kind="ExternalInput").ap()
    maskmin = nc.dram_tensor("maskmin", (4, P, QC), BF16, kind="ExternalInput").ap()
    out = nc.dram_tensor("out", (S, D), BF16, kind="ExternalOutput").ap()

    with tile.TileContext(nc) as tc:
        with (
            tc.tile_pool(name="sb", bufs=1) as sb,
            tc.tile_pool(name="ps", bufs=1, space="PSUM") as ps,
        ):
            # ---- persistent SBUF ----
            wq_sb = sb.tile([P, 8, 256], BF16, tag="wq")
            wk_sb = sb.tile([P, 8, 256], BF16, tag="wk")
            wv_sb = sb.tile([P, 8, 256], BF16, tag="wv")
            wo_sb = sb.tile([P, 2, D], BF16, tag="wo")
            mm_sb = sb.tile([P, 4, QC], BF16, tag="mask")
            QT = [sb.tile([69, S], BF16, tag=f"qt{s}", name=f"QT{s}") for s in range(4)]
            KT = [sb.tile([69, S], BF16, tag=f"kt{s}", name=f"KT{s}") for s in range(4)]
            Vb = sb.tile([P, NKT, 4, VW], BF16, tag="vb")
            OTs = [sb.tile([P, S], BF16, tag=f"ots{h}", name=f"OTs{h}") for h in range(2)]
            xts = [
                [
                    sb.tile([P, 4, QC], BF16, tag="xt", bufs=8,
                            name=f"xt_{q}_{h}")
                    for h in range(2)
                ]
                for q in range(NQC)
            ]

            # x^T in (quarter, dm-half) pieces, split across both DMA queues;
            # Q/K weights race the first x slices so the first MM starts early.
            nc.sync.dma_start(out=wq_sb, in_=wq.rearrange("(a p) m -> p a m", p=P))
            nc.gpsimd.dma_start(out=xts[0][1], in_=xTd[0, 1])
            nc.sync.dma_start(out=wk_sb, in_=wk.rearrange("(a p) m -> p a m", p=P))
            for q in range(NQC):
                nc.sync.dma_start(out=xts[q][0], in_=xTd[q, 0])
            nc.gpsimd.dma_start(out=wv_sb, in_=wv.rearrange("(a p) m -> p a m", p=P))
            for q in range(1, NQC):
                nc.gpsimd.dma_start(out=xts[q][1], in_=xTd[q, 1])
            nc.gpsimd.dma_start(out=wo_sb, in_=wo.rearrange("(h p) d -> p h d", p=P))
            nc.gpsimd.dma_start(out=mm_sb, in_=maskmin.rearrange("j p q -> p j q"))
            for s in range(4):
                nc.gpsimd.dma_start(out=QT[s][64:69, :], in_=qaug[s])
                nc.gpsimd.dma_start(out=KT[s][64:69, :], in_=kaug[s])
            # ones everywhere; V data columns overwritten below, leaving the
            # per-(tile,slot) ones column that yields the softmax denominator
            nc.vector.memset(Vb, 1.0)

            def proj_quarter(q):
                cs = slice(QC * q, QC * (q + 1))
                stq = ps.tile([P, 2, QC], F32, tag="st", bufs=2, name=f"pjq_{q}")
                stk = ps.tile([P, 2, QC], F32, tag="st", bufs=2, name=f"pjk_{q}")
                for k in range(8):
                    xsl = xts[q][k // 4][:, k % 4, :]
                    for st, w_sb in ((stq, wq_sb), (stk, wk_sb)):
                        for g in range(2):
                            nc.tensor.matmul(
                                st[:, g, :],
                                w_sb[:, k, 128 * g: 128 * (g + 1)],
                                xsl,
                                start=(k == 0), stop=(k == 7),
                            )
                for st, T in ((stq, QT), (stk, KT)):
                    for g in range(2):
                        nc.vector.tensor_copy(
                            out=T[2 * g][0:64, cs], in_=st[0:64, g, :]
                        )
                        stg = sb.tile([P, QC], BF16, tag="stg", bufs=2,
                                      name=f"stg_{q}_{id(T)}_{g}")
                        nc.scalar.copy(out=stg[64:128, :], in_=st[64:128, g, :])
                        nc.gpsimd.dma_start(
                            out=T[2 * g + 1][0:64, cs], in_=stg[64:128, :]
                        )
                # V in natural [s, d] layout: stationary = x^T s-slice
                for j in range(4):
                    sti = 4 * q + j
                    vt = ps.tile([P, 256], F32, tag="ot", bufs=4, name=f"v_{sti}")
                    for k in range(8):
                        nc.tensor.matmul(
                            vt,
                            xts[q][k // 4][:, k % 4, P * j: P * (j + 1)],
                            wv_sb[:, k, :],
                            start=(k == 0), stop=(k == 7),
                        )
                    nc.vector.tensor_copy(
                        out=Vb[:, sti, :, 0:HD],
                        in_=vt.rearrange("p (s d) -> p s d", s=4),
                    )

            pending = []  # deferred O-projection emitters (PE fill work)

            def pop_fill():
                if pending:
                    pending.pop(0)()

            def oproj_half(sti, n):
                def emit():
                    op = ps.tile([P, QC], F32, tag="st", bufs=2,
                                 name=f"op_{sti}_{n}")
                    for h in range(2):
                        nc.tensor.matmul(
                            op,
                            OTs[h][:, P * sti: P * (sti + 1)],
                            wo_sb[:, h, QC * n: QC * (n + 1)],
                            start=(h == 0), stop=(h == 1),
                        )
                    ost = sb.tile([P, QC], BF16, tag="ost", bufs=4,
                                  name=f"os_{sti}_{n}")
                    if n == 0:
                        nc.scalar.copy(out=ost, in_=op)
                    else:
                        nc.vector.tensor_copy(out=ost, in_=op)
                    nc.sync.dma_start(
                        out=out[P * sti: P * (sti + 1), QC * n: QC * (n + 1)],
                        in_=ost,
                    )
                return emit

            def attn_chunk(i):
                cs = slice(QC * i, QC * (i + 1))
                for a, b in ((0, 3), (1, 2)):
                    tl = {s: _tiles_for_chunk(i, s) for s in (a, b)}
                    otp = {
                        s: ps.tile([P, QC], F32, tag="ot", bufs=4,
                                   name=f"ot_{i}_{s}")
                        for s in (a, b)
                    }
                    def finalize(s):
                        recf = sb.tile([P, QC], F32, tag="recf", bufs=2,
                                       name=f"recf_{i}_{s}")
                        rec = sb.tile([P, QC], BF16, tag="rec", bufs=2,
                                      name=f"rec_{i}_{s}")
                        nc.scalar.activation(
                            out=recf[64:128, :], in_=otp[s][64:128, :],
                            func=mybir.ActivationFunctionType.Ln,
                        )
                        nc.scalar.activation(
                            out=rec[64:128, :], in_=recf[64:128, :],
                            func=mybir.ActivationFunctionType.Exp, scale=-1.0,
                        )
                        rec2 = sb.tile([64, QC], BF16, tag="rec2", bufs=2,
                                       name=f"rec2_{i}_{s}")
                        nc.gpsimd.dma_start(out=rec2, in_=rec[64:128, :])
                        h, half = s // 2, s % 2
                        if half == 0:
                            nc.vector.tensor_tensor(
                                out=OTs[h][0:64, cs], in0=otp[s][0:64, :],
                                in1=rec2, op=mybir.AluOpType.mult,
                            )
                        else:
                            ott = sb.tile([64, QC], BF16, tag="ott", bufs=2,
                                          name=f"ott_{i}_{s}")
                            nc.vector.tensor_tensor(
                                out=ott, in0=otp[s][0:64, :], in1=rec2,
                                op=mybir.AluOpType.mult,
                            )
                            nc.gpsimd.dma_start(out=OTs[h][64:128, cs], in_=ott)

                    ga, gb = _groups(tl[a]), _groups(tl[b])
                    rr = []
                    for z in range(max(len(ga), len(gb))):
                        if z < len(ga):
                            rr.append((a, ga[z], z == len(ga) - 1))
                        if z < len(gb):
                            rr.append((b, gb[z], z == len(gb) - 1))
                    for gz, (s, grp, last) in enumerate(rr):
                        st = ps.tile([P, 2, QC], F32, tag="st", bufs=2,
                                     name=f"st_{i}_{s}_{grp[0]}")
                        for u, t in enumerate(grp):
                            nc.tensor.matmul(
                                st[:, u, :],
                                KT[s][0:69, P * t: P * (t + 1)],
                                QT[s][0:69, cs],
                                start=True, stop=True,
                            )
                        pt = sb.tile([P, 2, QC], BF16, tag="pt", bufs=6,
                                     name=f"pt_{i}_{s}_{grp[0]}")
                        nc.scalar.activation(
                            out=pt, in_=st,
                            func=mybir.ActivationFunctionType.Exp,
                            bias=0.0, scale=1.0,
                        )
                        j = grp[0] - 4 * i
                        if j >= 0:
                            # diagonal pair: zero k>q via min with {1e9, 0}
                            nc.vector.tensor_tensor(
                                out=pt, in0=pt, in1=mm_sb[:, j: j + 2, :],
                                op=mybir.AluOpType.min,
                            )
                        for u, t in enumerate(grp):
                            nc.tensor.matmul(
                                otp[s],
                                Vb[:, t, s, :],
                                pt[:, u, :],
                                start=(t == tl[s][0]), stop=(t == tl[s][-1]),
                            )
                        if gz >= 2:
                            pop_fill()
                        if last:
                            finalize(s)
                for j in range(4):
                    sti = 4 * i + j
                    pending.append(oproj_half(sti, 0))
                    pending.append(oproj_half(sti, 1))

            for q in range(NQC):
                proj_quarter(q)
            for i in range(NQC):
                attn_chunk(i)
            while pending:
                pending.pop(0)()
    nc.compile()
    return nc


_NC_CACHE = None


def _get_graph():
    global _NC_CACHE
    if _NC_CACHE is None:
        _NC_CACHE = build_graph()
    return _NC_CACHE


def _host_inputs(x, W_Q, W_K, W_V, W_O):
    """Per-core input maps."""
    x = np.asarray(x, dtype=np.float32)
    W_Q = np.asarray(W_Q, dtype=np.float32)
    W_K = np.asarray(W_K, dtype=np.float32)
    W_V = np.asarray(W_V, dtype=np.float32)
    W_O = np.asarray(W_O, dtype=np.float32)

    xT = ml_dtypes.bfloat16(np.ascontiguousarray(x.transpose(0, 2, 1)))  # [B,D,S]
    # pre-tile to [NQC, dm-half, partition, dm-block, q] so each (quarter,
    # half) DMA reads contiguous 4KB lines per partition
    xTq = np.ascontiguousarray(
        xT.reshape(B, 2, 4, P, NQC, QC).transpose(0, 4, 1, 3, 2, 5)
    )

    karr = np.arange(S, dtype=np.float64)
    kk = np.float32(karr % P)                  # 0..127, exact in bf16
    qh = np.float32((karr % QC) // 32)         # 0..15, exact
    ql = np.float32((karr % QC) % 32)          # 0..31, exact
    ones = np.ones(S, dtype=np.float32)

    mask = np.zeros((4, P, QC), dtype=np.float64)
    kki = np.arange(P)[:, None]
    qqi = np.arange(QC)[None, :]
    for j in range(4):
        mask[j] = np.where(kki + P * j <= qqi, 1e9, 0.0)
    maskmin = ml_dtypes.bfloat16(mask)

    in_maps = []
    for c in range(NCORES):
        b, j = divmod(c, 4)
        heads = [j, 4 + j, 8 + j, 12 + j]
        rows = np.concatenate(
            [np.arange(64 * h, 64 * h + 64) for h in heads]
        )
        wq = ml_dtypes.bfloat16(np.ascontiguousarray((W_Q[rows, :] / 8.0).T))
        wk = ml_dtypes.bfloat16(np.ascontiguousarray(W_K[rows, :].T))
        wv = ml_dtypes.bfloat16(np.ascontiguousarray(W_V[rows, :].T))
        wo = ml_dtypes.bfloat16(np.ascontiguousarray(W_O[:, rows].T))

        kaug = np.zeros((4, 5, S), dtype=np.float32)
        qaug = np.zeros((4, 5, S), dtype=np.float32)
        for s, h in enumerate(heads):
            sl = float(SLOPES[h])
            # sum of exact-in-bf16 products = sl * (k - q):
            #   kk*sl + (k-kk)*sl - 32sl*qh - sl*ql - sl*(q - q%512)
            kaug[s, 0] = kk
            kaug[s, 1] = np.float32(karr - (karr % P))
            kaug[s, 2] = np.float32(-32.0 * sl)
            kaug[s, 3] = np.float32(-sl)
            kaug[s, 4] = np.float32(-sl)
            qaug[s, 0] = np.float32(sl) * ones
            qaug[s, 1] = np.float32(sl) * ones
            qaug[s, 2] = qh
            qaug[s, 3] = ql
            qaug[s, 4] = np.float32(karr - (karr % QC))
        in_maps.append(
            {
                "xT": xTq[b],
                "wq": wq,
                "wk": wk,
                "wv": wv,
                "wo": wo,
                "kaug": ml_dtypes.bfloat16(kaug),
                "qaug": ml_dtypes.bfloat16(qaug),
                "maskmin": maskmin,
            }
        )
    return in_maps


LAST_RESULTS = None


def kernel(x, W_Q, W_K, W_V, W_O):
    global LAST_RESULTS
    nc = _get_graph()
    in_maps = _host_inputs(x, W_Q, W_K, W_V, W_O)
    res = run_bass_kernel_spmd(nc, in_maps, core_ids=list(range(NCORES)))
    LAST_RESULTS = res
    total = np.zeros((B, S, D), dtype=np.float32)
    for c, r in enumerate(res.results):
        total[c // 4] += np.asarray(r["out"], dtype=np.float32)
    return total


if __name__ == "__main__":
    nc = build_graph()
    print("graph built ok")


# revision 15
# speedup vs baseline: 1.3012x; 1.0409x over previous
"""ALiBi causal attention (B=2, S=2048, D=1024, H=16) on 8 TRN2 NeuronCores.

Sharding: core c owns batch c//4 and heads {j, 4+j, 8+j, 12+j} (j = c%4),
one head per "slot" 0..3. All cores run the same graph (SPMD); slot s uses
the ALiBi window of its shallowest head (h = 4s+3), so per-slot k-tile
lists are core-independent.

Per core:
  - x^T streamed in 4 sequence-quarters; Q^T/K^T projected with quarter-wide
    matmuls (4 psum sub-slices of one 4-bank tile), V projected directly in
    natural [s, d] layout (stationary = x^T slice) - no PE transposes.
  - windowed causal flash-attention per slot, two slot-streams round-robin
    per q-chunk; ALiBi bias folded into the scores matmul via 5 exact
    bf16 "augmented" rows; exp on ScalarE in quad-tile ACTIVATEs
    ([128,4,512], amortizes the ~352-cycle fixed cost); diagonal quad
    masked post-exp by a min with {1e9,0}; P^T@V accumulated with a
    ones-column in V producing the softmax denominator; 1/rowsum via DVE
    reciprocal_approx_fast on the replicated rows, DMA-shifted to base 0.
  - W_O slice applied per 128-row tile, woven between attention groups of
    the next chunk to fill PE gaps while ScalarE runs exp.
Host sums the 4 partial outputs per batch (the W_O reduce) and reshapes.
"""

import math
import sys

sys.path.insert(0, "/opt/trn_rl_repo")

import ml_dtypes
import numpy as np

import concourse.bass as bass
import concourse.tile as tile
from concourse import bacc, mybir
from concourse.bass_utils import run_bass_kernel_spmd

B, S, D, H, HD = 2, 2048, 1024, 16, 64
P = 128          # k-tile / partition size
QC = 512         # q-chunk size
NKT = S // P     # 16 k-tiles
NQC = S // QC    # 4 q-chunks
NCORES = 8
T_WIN = 12.0     # ln cutoff: drop k-tiles with alibi penalty > T_WIN
VW = P           # V cols per (k-tile, slot): 64 data + 64 ones (the ones
                 # block makes P^T@V replicate the softmax denominator into
                 # psum rows 64:128, so no partition-broadcast is needed)

F32 = mybir.dt.float32
BF16 = mybir.dt.bfloat16

# slopes m_h = 2^-(h+1)/2, snapped to bf16 so every aug-row value is exactly
# representable regardless of PE input rounding.
SLOPES = np.float32(ml_dtypes.bfloat16(2.0 ** (-(np.arange(H, dtype=np.float64) + 1) / 2.0)))

# slot s covers heads 4s..4s+3 across the 4 cores of a batch; its k-tile
# window is set by the shallowest member (h = 4s+3). Slot 3 is full causal.
SLOT_L = [T_WIN / float(SLOPES[4 * s + 3]) for s in range(3)] + [None]


def _tiles_for_chunk(i: int, s: int):
    """Off-diagonal k-tiles kept for q-chunk i of slot s (identical on all
    cores). The 4 diagonal tiles are handled separately as 128x128 blocks."""
    if SLOT_L[s] is None:
        lo = 0
    else:
        lo = max(0, math.ceil((QC * i - SLOT_L[s] - (P - 1)) / P))
        if (4 * i - lo) % 2:  # keep counts even for pair grouping
            lo = max(0, lo - 1)
    return list(range(lo, 4 * i))


# diagonal 128x128 blocks (k_tile_offset j, q_sub) for the two exp groups,
# masked blocks (j == q_sub) ordered last so one TT covers both
DIAG_GROUPS = [
    [(0, 2), (1, 2), (0, 0), (2, 2)],          # subs {0,2}: masked (0,0),(2,2)
    [(0, 1), (0, 3), (1, 3), (2, 3), (1, 1), (3, 3)],  # subs {1,3}
]


def _groups(lst):
    """Split an even-length ascending tile list into pairs."""
    out = []
    pos = 0
    while pos < len(lst):
        out.append(lst[pos:pos + 2])
        pos += 2
    return out


def _only_combined_act_table(arch):
    """Restrict the ACT LUT chooser so exp/copy resolve only to the one set
    holding them all - the Scalar engine never reloads tables mid-kernel."""
    from concourse import mybir as _mb
    from concourse.hw_specs import get_activation_tables as _orig
    tabs = _orig(arch)
    keep = "natural_log_exp_and_others"
    mine = {_mb.ActivationFunctionType.Exp, _mb.ActivationFunctionType.Ln,
            _mb.ActivationFunctionType.Copy, _mb.ActivationFunctionType.Identity}
    return {
        name: (fns if name == keep else (set(fns) - mine))
        for name, fns in tabs.items()
    }


def build_graph() -> bass.Bass:
    bacc.get_activation_tables = _only_combined_act_table
    nc = bacc.Bacc("TRN2", target_bir_lowering=False, debug=False)

    xTd = nc.dram_tensor("xT", (NQC, 2, P, 4, QC), BF16, kind="ExternalInput").ap()
    wq = nc.dram_tensor("wq", (D, 256), BF16, kind="ExternalInput").ap()
    wk = nc.dram_tensor("wk", (D, 256), BF16, kind="ExternalInput").ap()
    wv = nc.dram_tensor("wv", (D, 256), BF16, kind="ExternalInput").ap()
    wo = nc.dram_tensor("wo", (256, D), BF16, kind="ExternalInput").ap()
    kaug = nc.dram_tensor("kaug", (4, 5, S), BF16, kind="ExternalInput").ap()
    qaug = nc.dram_tensor("qaug", (4, 5, S), BF16, kind="ExternalInput").ap()
    maskmin = nc.dram_tensor("maskmin", (2, P, P), BF16, kind="ExternalInput").ap()
    out = nc.dram_tensor("out", (S, D), BF16, kind="ExternalOutput").ap()

    with tile.TileContext(nc) as tc:
        with (
            tc.tile_pool(name="sb", bufs=1) as sb,
            tc.tile_pool(name="ps", bufs=1, space="PSUM") as ps,
        ):
            # ---- persistent SBUF ----
            wq_sb = sb.tile([P, 8, 256], BF16, tag="wq")
            wk_sb = sb.tile([P, 8, 256], BF16, tag="wk")
            wv_sb = sb.tile([P, 8, 256], BF16, tag="wv")
            wo_sb = sb.tile([P, 2, D], BF16, tag="wo")
            mm_sb = sb.tile([P, 2, P], BF16, tag="mask")
            QT = [sb.tile([69, S], BF16, tag=f"qt{s}", name=f"QT{s}") for s in range(4)]
            KT = [sb.tile([69, S], BF16, tag=f"kt{s}", name=f"KT{s}") for s in range(4)]
            Vb = sb.tile([P, NKT, 4, VW], BF16, tag="vb")
            OTs = [sb.tile([P, S], BF16, tag=f"ots{h}", name=f"OTs{h}") for h in range(2)]
            xts = [
                [
                    sb.tile([P, 4, QC], BF16, tag="xt", bufs=8,
                            name=f"xt_{q}_{h}")
                    for h in range(2)
                ]
                for q in range(NQC)
            ]

            # x^T in (quarter, dm-half) pieces, split across both DMA queues;
            # Q/K weights race the first x slices so the first MM starts early.
            nc.sync.dma_start(out=wq_sb, in_=wq.rearrange("(a p) m -> p a m", p=P))
            nc.sync.dma_start(out=xts[0][0], in_=xTd[0, 0])
            nc.gpsimd.dma_start(out=xts[0][1], in_=xTd[0, 1])
            nc.sync.dma_start(out=wk_sb, in_=wk.rearrange("(a p) m -> p a m", p=P))
            for q in range(1, NQC):
                nc.sync.dma_start(out=xts[q][0], in_=xTd[q, 0])
            nc.gpsimd.dma_start(out=wv_sb, in_=wv.rearrange("(a p) m -> p a m", p=P))
            for q in range(1, NQC):
                nc.gpsimd.dma_start(out=xts[q][1], in_=xTd[q, 1])
            nc.gpsimd.dma_start(out=wo_sb, in_=wo.rearrange("(h p) d -> p h d", p=P))
            nc.gpsimd.dma_start(out=mm_sb, in_=maskmin.rearrange("u p q -> p u q"))
            for s in range(4):
                nc.gpsimd.dma_start(out=QT[s][64:69, :], in_=qaug[s])
                nc.gpsimd.dma_start(out=KT[s][64:69, :], in_=kaug[s])
            # ones everywhere; V data columns overwritten below, leaving the
            # per-(tile,slot) ones column that yields the softmax denominator
            nc.vector.memset(Vb, 1.0)

            def proj_quarter(q):
                cs = slice(QC * q, QC * (q + 1))
                stq = ps.tile([P, 2, QC], F32, tag="st", bufs=2, name=f"pjq_{q}")
                stk = ps.tile([P, 2, QC], F32, tag="st", bufs=2, name=f"pjk_{q}")
                for st, w_sb in ((stq, wq_sb), (stk, wk_sb)):
                    for k in range(8):
                        for g in range(2):
                            nc.tensor.matmul(
                                st[:, g, :],
                                w_sb[:, k, 128 * g: 128 * (g + 1)],
                                xts[q][k // 4][:, k % 4, :],
                                start=(k == 0), stop=(k == 7),
                            )
                for st, T in ((stq, QT), (stk, KT)):
                    for g in range(2):
                        nc.vector.tensor_copy(
                            out=T[2 * g][0:64, cs], in_=st[0:64, g, :]
                        )
                        stg = sb.tile([P, QC], BF16, tag="stg", bufs=2,
                                      name=f"stg_{q}_{id(T)}_{g}")
                        nc.scalar.copy(out=stg[64:128, :], in_=st[64:128, g, :])
                        nc.gpsimd.dma_start(
                            out=T[2 * g + 1][0:64, cs], in_=stg[64:128, :]
                        )
                # V in natural [s, d] layout: stationary = x^T s-slice
                for j in range(4):
                    sti = 4 * q + j
                    vt = ps.tile([P, 256], F32, tag="ot", bufs=4, name=f"v_{sti}")
                    for k in range(8):
                        nc.tensor.matmul(
                            vt,
                            xts[q][k // 4][:, k % 4, P * j: P * (j + 1)],
                            wv_sb[:, k, :],
                            start=(k == 0), stop=(k == 7),
                        )
                    nc.vector.tensor_copy(
                        out=Vb[:, sti, :, 0:HD],
                        in_=vt.rearrange("p (s d) -> p s d", s=4),
                    )

            pending = []  # deferred O-projection emitters (PE fill work)

            def pop_fill():
                if pending:
                    pending.pop(0)()

            def oproj_half(sti, n):
                def emit():
                    op = ps.tile([P, QC], F32, tag="st", bufs=2,
                                 name=f"op_{sti}_{n}")
                    for h in range(2):
                        nc.tensor.matmul(
                            op,
                            OTs[h][:, P * sti: P * (sti + 1)],
                            wo_sb[:, h, QC * n: QC * (n + 1)],
                            start=(h == 0), stop=(h == 1),
                        )
                    ost = sb.tile([P, QC], BF16, tag="ost", bufs=4,
                                  name=f"os_{sti}_{n}")
                    nc.vector.tensor_copy(out=ost, in_=op)
                    nc.sync.dma_start(
                        out=out[P * sti: P * (sti + 1), QC * n: QC * (n + 1)],
                        in_=ost,
                    )
                return emit

            def attn_chunk(i):
                cs = slice(QC * i, QC * (i + 1))
                for a, b in ((0, 3), (1, 2)):
                    tl = {s: _tiles_for_chunk(i, s) for s in (a, b)}
                    otp = {
                        s: ps.tile([P, QC], F32, tag="ot", bufs=4,
                                   name=f"ot_{i}_{s}")
                        for s in (a, b)
                    }
                    def finalize(s):
                        recf = sb.tile([P, QC], F32, tag="recf", bufs=2,
                                       name=f"recf_{i}_{s}")
                        rec = sb.tile([P, QC], BF16, tag="rec", bufs=2,
                                      name=f"rec_{i}_{s}")
                        nc.scalar.activation(
                            out=recf[64:128, :], in_=otp[s][64:128, :],
                            func=mybir.ActivationFunctionType.Ln,
                        )
                        nc.scalar.activation(
                            out=rec[64:128, :], in_=recf[64:128, :],
                            func=mybir.ActivationFunctionType.Exp, scale=-1.0,
                        )
                        rec2 = sb.tile([64, QC], BF16, tag="rec2", bufs=2,
                                       name=f"rec2_{i}_{s}")
                        nc.gpsimd.dma_start(out=rec2, in_=rec[64:128, :])
                        h, half = s // 2, s % 2
                        if half == 0:
                            nc.vector.tensor_tensor(
                                out=OTs[h][0:64, cs], in0=otp[s][0:64, :],
                                in1=rec2, op=mybir.AluOpType.mult,
                            )
                        else:
                            ott = sb.tile([64, QC], BF16, tag="ott", bufs=2,
                                          name=f"ott_{i}_{s}")
                            nc.vector.tensor_tensor(
                                out=ott, in0=otp[s][0:64, :], in1=rec2,
                                op=mybir.AluOpType.mult,
                            )
                            nc.gpsimd.dma_start(out=OTs[h][64:128, cs], in_=ott)

                    def offdiag_group(s, grp, first):
                        st = ps.tile([P, 2, QC], F32, tag="st", bufs=2,
                                     name=f"st_{i}_{s}_{grp[0]}")
                        for u, t in enumerate(grp):
                            nc.tensor.matmul(
                                st[:, u, :],
                                KT[s][0:69, P * t: P * (t + 1)],
                                QT[s][0:69, cs],
                                start=True, stop=True,
                            )
                        pt = sb.tile([P, 2, QC], BF16, tag="pt", bufs=6,
                                     name=f"pt_{i}_{s}_{grp[0]}")
                        nc.scalar.activation(
                            out=pt, in_=st,
                            func=mybir.ActivationFunctionType.Exp,
                            bias=0.0, scale=1.0,
                        )
                        for u, t in enumerate(grp):
                            nc.tensor.matmul(
                                otp[s],
                                Vb[:, t, s, :],
                                pt[:, u, :],
                                start=first and u == 0, stop=False,
                            )

                    def diag_group(s, gi):
                        # start=True clears has_written for the WHOLE psum
                        # bank - emit it only on the very first matmul into
                        # otp; per-element flags then handle region firsts
                        grp = DIAG_GROUPS[gi]
                        n = len(grp)
                        nodiag = len(tl[s]) == 0
                        st = ps.tile([P, n, P], F32, tag="st", bufs=2,
                                     name=f"std_{i}_{s}_{gi}")
                        for u, (j, sub) in enumerate(grp):
                            nc.tensor.matmul(
                                st[:, u, :],
                                KT[s][0:69, P * (4 * i + j): P * (4 * i + j + 1)],
                                QT[s][0:69, QC * i + P * sub: QC * i + P * (sub + 1)],
                                start=True, stop=True,
                            )
                        pt = sb.tile([P, 6, P], BF16, tag="ptd", bufs=4,
                                     name=f"ptd_{i}_{s}_{gi}")
                        nc.scalar.activation(
                            out=pt[:, 0:n, :], in_=st,
                            func=mybir.ActivationFunctionType.Exp,
                            bias=0.0, scale=1.0,
                        )
                        # the last two blocks are the half-masked diagonals
                        nc.vector.tensor_tensor(
                            out=pt[:, n - 2: n, :], in0=pt[:, n - 2: n, :],
                            in1=mm_sb, op=mybir.AluOpType.min,
                        )
                        for u, (j, sub) in enumerate(grp):
                            nc.tensor.matmul(
                                otp[s][:, P * sub: P * (sub + 1)],
                                Vb[:, 4 * i + j, s, :],
                                pt[:, u, :],
                                start=nodiag and gi == 0 and u == 0,
                                stop=j == sub,
                            )

                    rr = []
                    for s, other in ((a, b), (b, a)):
                        gs = [("o", g) for g in _groups(tl[s])]
                        gs += [("d", 0), ("d", 1)]
                        rr.append([(s, kind, g) for kind, g in gs])
                    seq = []
                    ga, gb = rr
                    for z in range(max(len(ga), len(gb))):
                        if z < len(ga):
                            seq.append(ga[z] + (z == len(ga) - 1,))
                        if z < len(gb):
                            seq.append(gb[z] + (z == len(gb) - 1,))
                    for gz, (s, kind, g, last) in enumerate(seq):
                        if kind == "o":
                            offdiag_group(s, g, g[0] == tl[s][0])
                        else:
                            diag_group(s, g)
                        if gz >= 2:
                            pop_fill()
                        if last:
                            finalize(s)
                for j in range(4):
                    sti = 4 * i + j
                    pending.append(oproj_half(sti, 0))
                    pending.append(oproj_half(sti, 1))

            for q in range(NQC):
                proj_quarter(q)
            for i in range(NQC):
                attn_chunk(i)
            while pending:
                pending.pop(0)()
    nc.compile()
    return nc


_NC_CACHE = None


def _get_graph():
    global _NC_CACHE
    if _NC_CACHE is None:
        _NC_CACHE = build_graph()
    return _NC_CACHE


def _host_inputs(x, W_Q, W_K, W_V, W_O):
    """Per-core input maps."""
    x = np.asarray(x, dtype=np.float32)
    W_Q = np.asarray(W_Q, dtype=np.float32)
    W_K = np.asarray(W_K, dtype=np.float32)
    W_V = np.asarray(W_V, dtype=np.float32)
    W_O = np.asarray(W_O, dtype=np.float32)

    xT = ml_dtypes.bfloat16(np.ascontiguousarray(x.transpose(0, 2, 1)))  # [B,D,S]
    # pre-tile to [NQC, dm-half, partition, dm-block, q] so each (quarter,
    # half) DMA reads contiguous 4KB lines per partition
    xTq = np.ascontiguousarray(
        xT.reshape(B, 2, 4, P, NQC, QC).transpose(0, 4, 1, 3, 2, 5)
    )

    karr = np.arange(S, dtype=np.float64)
    kk = np.float32(karr % P)                  # 0..127, exact in bf16
    qh = np.float32((karr % QC) // 32)         # 0..15, exact
    ql = np.float32((karr % QC) % 32)          # 0..31, exact
    ones = np.ones(S, dtype=np.float32)

    kki = np.arange(P)[:, None]
    qqi = np.arange(P)[None, :]
    tri = np.where(kki <= qqi, 1e9, 0.0)
    maskmin = ml_dtypes.bfloat16(np.stack([tri, tri]))

    in_maps = []
    for c in range(NCORES):
        b, j = divmod(c, 4)
        heads = [j, 4 + j, 8 + j, 12 + j]
        rows = np.concatenate(
            [np.arange(64 * h, 64 * h + 64) for h in heads]
        )
        wq = ml_dtypes.bfloat16(np.ascontiguousarray((W_Q[rows, :] / 8.0).T))
        wk = ml_dtypes.bfloat16(np.ascontiguousarray(W_K[rows, :].T))
        wv = ml_dtypes.bfloat16(np.ascontiguousarray(W_V[rows, :].T))
        wo = ml_dtypes.bfloat16(np.ascontiguousarray(W_O[:, rows].T))

        kaug = np.zeros((4, 5, S), dtype=np.float32)
        qaug = np.zeros((4, 5, S), dtype=np.float32)
        for s, h in enumerate(heads):
            sl = float(SLOPES[h])
            # sum of exact-in-bf16 products = sl * (k - q):
            #   kk*sl + (k-kk)*sl - 32sl*qh - sl*ql - sl*(q - q%512)
            kaug[s, 0] = kk
            kaug[s, 1] = np.float32(karr - (karr % P))
            kaug[s, 2] = np.float32(-32.0 * sl)
            kaug[s, 3] = np.float32(-sl)
            kaug[s, 4] = np.float32(-sl)
            qaug[s, 0] = np.float32(sl) * ones
            qaug[s, 1] = np.float32(sl) * ones
            qaug[s, 2] = qh
            qaug[s, 3] = ql
            qaug[s, 4] = np.float32(karr - (karr % QC))
        in_maps.append(
            {
                "xT": xTq[b],
                "wq": wq,
                "wk": wk,
                "wv": wv,
                "wo": wo,
                "kaug": ml_dtypes.bfloat16(kaug),
                "qaug": ml_dtypes.bfloat16(qaug),
                "maskmin": maskmin,
            }
        )
    return in_maps


LAST_RESULTS = None


def kernel(x, W_Q, W_K, W_V, W_O):
    global LAST_RESULTS
    nc = _get_graph()
    in_maps = _host_inputs(x, W_Q, W_K, W_V, W_O)
    res = run_bass_kernel_spmd(nc, in_maps, core_ids=list(range(NCORES)))
    LAST_RESULTS = res
    total = np.zeros((B, S, D), dtype=np.float32)
    for c, r in enumerate(res.results):
        total[c // 4] += np.asarray(r["out"], dtype=np.float32)
    return total


if __name__ == "__main__":
    nc = build_graph()
    print("graph built ok")
